# revision 20
# baseline (speedup 1.0000x reference)
"""CTRGC fused kernel for Trainium2, 8-core data-parallel over batch.

Reference math (N,C,O,R,T,V = 128,64,64,8,128,25):
  x1 = mean_t(w1@x+b1); x2 = mean_t(w2@x+b2); x3 = w5@x+b5
  gate: g = mean_tv(x3); p = mean_v(x3); gg = g_w@g+g_b
        h = relu(BN(tconv3(p+gg, a_w)+a_b)); rf = tconv3(h, rf_bw)+1
  z   = w3@(x3*rf)+b3                                  [N,O,T,V]
  rel = w4@tanh(x1[:,:,u]-x2[:,:,v])*alpha + b4*alpha + A   [N,O,V,V]
  out[n,o,t,u] = sum_v rel[n,o,u,v] * z[n,o,t,v]

Per-core layouts (NB=16 batch items per core):
  x_sb   [128=(jj*64+c), 1600=(th,v)]    jj = t-half, th = t % 64
  conv-gate pipeline on partitions (jj*32+r), i.e. rows 0:8 and 32:40
  y      [40=(jj,r), 1600=(v,th)]
  yT     [104=(r,v<13), 128=t] + [96=(r,v>=13), 128=t]  (one DMA each)
  zT     [125=(og,v), 13*128=(g,t)] built on PE: per o-group g,
         zT_g = W1_g @ yTA + W2_g @ yTB with host-precomputed
         block-expanded w3 weights (W[(r,v'),(og,v)] = w3[5g+og,r] d(v'=v))
  BD     [125, 13*125] block-diag rel3, built by diagonal-AP DMAs
  final  out_psum[t, (o_g,u)] = zT_g.T @ BD_g  per group g of 5 o's
"""
import numpy as np

import concourse.bacc as bacc
import concourse.mybir as mybir
import concourse.tile as tile
from concourse.ap import AP
from concourse import bass_utils

f32 = mybir.dt.float32
bf16 = mybir.dt.bfloat16
NCORES = 8
N, C, O, R, T, V = 128, 64, 64, 8, 128, 25
NB = N // NCORES      # batch per core
TH = T // 2           # t-half
VTH = V * TH
EPS = 1e-5
VCH = [(0, 6), (6, 12), (12, 18), (18, 25)]  # v-chunks
NG = 13               # o-groups of 5 (last has 4)
VA = 13               # v-split for yT tiles: A has v<13, B has v>=13

AF = mybir.ActivationFunctionType
ALU = mybir.AluOpType
AXL = mybir.AxisListType

BF16_CONSTS = ("cw5x3", "cwz1", "cwz2")
CONST_SPECS = [
    ("cw5x3", (128, 40)), ("cw5p", (128, 40)), ("cw12", (128, 40)),
    ("cb1", (8, 1)), ("cb2", (8, 1)), ("cgw", (40, 40)), ("cgb", (40, 1)), ("cb5", (40, 1)),
    ("caw0", (40, 40)), ("caw1", (40, 40)), ("caw2", (40, 40)), ("cc0", (40, 1)),
    ("crfw0", (40, 40)), ("crfw1", (40, 40)), ("crfw2", (40, 40)),
    ("cwz1", (8 * VA, NG * 125)), ("cwz2", (8 * (V - VA), NG * 125)),
    ("cb3bd", (125, NG)), ("cw4a", (10, 64)), ("carel", (2, 625)),
]


def host_consts(A_, alpha, w1, b1, w2, b2, w5, b5, w3, b3, w4, b4,
                g_w, g_b, a_w, a_b, bn_g, bn_b, bn_rm, bn_rv, rf_bw):
    al = float(alpha[0])
    bn_scale = bn_g / np.sqrt(bn_rv + EPS)
    c0 = (a_b - bn_rm) * bn_scale + bn_b
    d = {}
    cw5x3 = np.zeros((128, 40), np.float32)
    cw5p = np.zeros((128, 40), np.float32)
    cw12 = np.zeros((128, 40), np.float32)
    for jj in range(2):
        cw5x3[jj * 64:(jj + 1) * 64, jj * 32:jj * 32 + 8] = w5.T
        for jj2 in range(2):
            cw5p[jj2 * 64:(jj2 + 1) * 64, jj * 32:jj * 32 + 8] = w5.T / V
        cw12[jj * 64:(jj + 1) * 64, 0:8] = w1.T / T
        cw12[jj * 64:(jj + 1) * 64, 32:40] = w2.T / T
    d["cw5x3"], d["cw5p"], d["cw12"] = cw5x3, cw5p, cw12

    def dup40(v8):
        o = np.zeros((40, 1), np.float32)
        o[0:8, 0] = v8
        o[32:40, 0] = v8
        return o

    def bd40(m8T):  # m8T [r'(row), r(col)] -> blockdiag [40,40]
        o = np.zeros((40, 40), np.float32)
        o[0:8, 0:8] = m8T
        o[32:40, 32:40] = m8T
        return o

    d["cb1"] = b1.reshape(8, 1)
    d["cb2"] = b2.reshape(8, 1)
    d["cgw"] = bd40(g_w.T / T)
    d["cgb"] = dup40(g_b)
    d["cb5"] = dup40(b5)
    for k in range(3):
        d[f"caw{k}"] = bd40((a_w[:, :, k, 0] * bn_scale[:, None]).T)
        d[f"crfw{k}"] = bd40(rf_bw[:, :, k, 0].T)
    d["cc0"] = dup40(c0)
    # block-expanded w3 for direct zT production on PE:
    # zT_g[(og,v), t] = sum_{(r,v')} W_g[(r,v'),(og,v)] yT[(r,v'), t]
    cwz1 = np.zeros((8 * VA, NG * 125), np.float32)
    cwz2 = np.zeros((8 * (V - VA), NG * 125), np.float32)
    cb3bd = np.zeros((125, NG), np.float32)
    rr = np.arange(8)
    for g in range(NG):
        ocnt = 5 if g < NG - 1 else 4
        for og in range(ocnt):
            o = g * 5 + og
            for v in range(V):
                col = g * 125 + og * 25 + v
                if v < VA:
                    cwz1[rr * VA + v, col] = w3[o]
                else:
                    cwz2[rr * (V - VA) + (v - VA), col] = w3[o]
            cb3bd[og * 25:(og + 1) * 25, g] = b3[o]
    d["cwz1"], d["cwz2"], d["cb3bd"] = cwz1, cwz2, cb3bd
    cw4a = np.zeros((10, 64), np.float32)
    cw4a[0:8] = (al * w4).T
    cw4a[8] = al * b4
    cw4a[9] = 1.0
    d["cw4a"] = cw4a
    carel = np.zeros((2, 625), np.float32)
    carel[0] = 1.0
    carel[1] = A_.T.reshape(625)  # [v*25+u] = A[u,v]
    d["carel"] = carel
    from ml_dtypes import bfloat16
    return {k: np.ascontiguousarray(v, bfloat16 if k in BF16_CONSTS else np.float32)
            for k, v in d.items()}


def build(nb=NB, upto=9):
    nc = bacc.Bacc("TRN2", target_bir_lowering=False, debug=False)
    x_d = nc.dram_tensor("x", [nb, C, T, V], f32, kind="ExternalInput")
    out_d = nc.dram_tensor("out", [nb, O, T, V], f32, kind="ExternalOutput")
    cdram = {nm: nc.dram_tensor(nm, list(sh),
                                bf16 if nm in BF16_CONSTS else f32,
                                kind="ExternalInput")
             for nm, sh in CONST_SPECS}

    with tile.TileContext(nc) as tc:
        with (
            tc.tile_pool(name="pconst", bufs=1) as pconst,
            tc.tile_pool(name="ppersist", bufs=1) as ppersist,
            tc.tile_pool(name="px", bufs=2) as px,
            tc.tile_pool(name="pbig", bufs=2) as pbig,
            tc.tile_pool(name="psmall", bufs=2) as psmall,
            tc.tile_pool(name="qz", bufs=2, space="PSUM") as qz,
            tc.tile_pool(name="qzt", bufs=2, space="PSUM") as qzt,
            tc.tile_pool(name="qout", bufs=1, space="PSUM") as qout,
            tc.tile_pool(name="qsm", bufs=2, space="PSUM") as qsm,
        ):
            cs = {}
            for nm, sh in CONST_SPECS:
                ct = pconst.tile(list(sh), bf16 if nm in BF16_CONSTS else f32,
                                 tag=nm)
                nc.sync.dma_start(ct[:], cdram[nm].ap())
                cs[nm] = ct

            # persistent: block-diag rel3 (off-diag zeros live forever),
            # rel' with const rows 8 (ones) / 9 (A_vu)
            bd = ppersist.tile([125, NG * 125], bf16, tag="bd")
            nc.vector.memset(bd[:], 0.0)
            relp = ppersist.tile([10, 625], f32, tag="relp")
            nc.sync.dma_start(relp[8:10, :], cdram["carel"].ap())

            bd_t = bd[:].tensor
            relp_t = relp[:].tensor

            # fixed ping-pong tiles for everything touched by custom-AP
            # DMAs (dynamic pool slots + diagonal APs confuse the race
            # tracking, so give these stable addresses)
            rel3_pp = [ppersist.tile([64, 625], bf16, tag=f"rel3_{i}", name=f"rel3pp{i}")
                       for i in range(2)]
            rel3g_pp = [ppersist.tile([125, NG * 25], bf16, tag=f"rel3g_{i}", name=f"rel3gpp{i}")
                        for i in range(2)]
            yTA_pp = [ppersist.tile([8 * VA, T], bf16, tag=f"yTA_{i}", name=f"yTApp{i}")
                      for i in range(2)]
            yTB_pp = [ppersist.tile([8 * (V - VA), T], bf16, tag=f"yTB_{i}", name=f"yTBpp{i}")
                      for i in range(2)]
            zT_pp = [ppersist.tile([125, NG * T], bf16, tag=f"zT_{i}", name=f"zTpp{i}")
                     for i in range(2)]
            out_pp = [ppersist.tile([128, O * V], f32, tag=f"out_{i}", name=f"outpp{i}")
                      for i in range(2)]

            for n in range(nb):
                # ---- load x [128=(jj,c), 1600=(th,v)] ----
                x_sb = px.tile([128, TH * V], f32, tag="x")
                nc.sync.dma_start(
                    x_sb[:],
                    AP(x_d.ap().tensor, n * C * T * V,
                       [[TH * V, 2], [T * V, C], [1, TH * V]]))
                x_t = x_sb[:].tensor

                # ---- reductions over v (for gate) and over th (for x1/x2) ----
                xv2 = psmall.tile([128, TH], f32, tag="xv2")  # sum over v
                nc.vector.tensor_reduce(
                    xv2[:], AP(x_t, 0, [[TH * V, 128], [V, TH], [1, V]]),
                    AXL.X, ALU.add)
                xt = psmall.tile([128, V], f32, tag="xt")     # sum over th
                nc.vector.tensor_reduce(
                    xt[:], AP(x_t, 0, [[TH * V, 128], [1, V], [V, TH]]),
                    AXL.X, ALU.add)

                if upto < 2:
                    nc.sync.dma_start(out_d.ap()[n].transpose([1, 0, 2]),
                                      x_sb[:].rearrange("p (a b) -> p a b", a=64))
                    continue
                # ---- gate chain on rows (jj*32+r) ----
                p_psa = qsm.tile([40, TH], f32, tag="sm", name=f"p_psa{n}")
                p_psb = qsm.tile([40, TH], f32, tag="sm", name=f"p_psb{n}")
                nc.tensor.matmul(p_psa[:], cs["cw5p"][0:64, :],
                                 xv2[0:64, :], start=True, stop=True)
                nc.tensor.matmul(p_psb[:], cs["cw5p"][64:128, :],
                                 xv2[64:128, :], start=True, stop=True)
                s_pre = psmall.tile([40, T], f32, tag="s_pre")
                g_rawa = psmall.tile([40, 1], f32, tag="g_rawa")
                g_rawb = psmall.tile([40, 1], f32, tag="g_rawb")
                g_raw = psmall.tile([40, 1], f32, tag="g_raw")
                nc.scalar.activation(s_pre[:, 0:TH], p_psa[:], AF.Identity,
                                     bias=cs["cb5"][:], accum_out=g_rawa[:])
                nc.scalar.activation(s_pre[:, TH:T], p_psb[:], AF.Identity,
                                     bias=cs["cb5"][:], accum_out=g_rawb[:])
                nc.vector.tensor_add(g_raw[:], g_rawa[:], g_rawb[:])
                gg_ps = qsm.tile([40, 1], f32, tag="sm")
                nc.tensor.matmul(gg_ps[:], cs["cgw"][:], g_raw[:],
                                 start=True, stop=True)
                gg_sb = psmall.tile([40, 1], f32, tag="gg_sb")
                nc.scalar.activation(gg_sb[:], gg_ps[:], AF.Identity,
                                     bias=cs["cgb"][:])
                s_sb = psmall.tile([40, T], f32, tag="s_sb")
                nc.vector.tensor_scalar_add(s_sb[:], s_pre[:], gg_sb[:])

                h_ps = qsm.tile([40, T], f32, tag="sm")
                nc.tensor.matmul(h_ps[:, 0:T], cs["caw1"][:], s_sb[:, 0:T],
                                 start=True, stop=False, skip_group_check=True)
                nc.tensor.matmul(h_ps[:, 1:T], cs["caw0"][:], s_sb[:, 0:T - 1],
                                 start=False, stop=False, skip_group_check=True)
                nc.tensor.matmul(h_ps[:, 0:T - 1], cs["caw2"][:], s_sb[:, 1:T],
                                 start=False, stop=True, skip_group_check=True)
                hr = psmall.tile([40, T], f32, tag="hr")
                nc.scalar.activation(hr[:], h_ps[:], AF.Relu, bias=cs["cc0"][:])

                rf_ps = qsm.tile([40, T], f32, tag="sm")
                nc.tensor.matmul(rf_ps[:, 0:T], cs["crfw1"][:], hr[:, 0:T],
                                 start=True, stop=False, skip_group_check=True)
                nc.tensor.matmul(rf_ps[:, 1:T], cs["crfw0"][:], hr[:, 0:T - 1],
                                 start=False, stop=False, skip_group_check=True)
                nc.tensor.matmul(rf_ps[:, 0:T - 1], cs["crfw2"][:], hr[:, 1:T],
                                 start=False, stop=True, skip_group_check=True)
                rf2 = psmall.tile([40, TH], f32, tag="rf2")
                nc.gpsimd.memset(rf2[:], 1.0)  # rows 8:32 stay 1.0 (dead lanes)
                nc.scalar.activation(rf2[0:8, :], rf_ps[0:8, 0:TH],
                                     AF.Identity, bias=1.0)
                nc.scalar.activation(rf2[32:40, :], rf_ps[32:40, TH:T],
                                     AF.Identity, bias=1.0)

                if upto < 3:
                    nc.sync.dma_start(out_d.ap()[n].transpose([1, 0, 2]),
                                      x_sb[:].rearrange("p (a b) -> p a b", a=64))
                    continue
                # ---- x1/x2 from the th-sum, two m=8 matmuls so both land
                # on partitions 0:8 (compute ops need same start partition) ----
                x1_ps = qsm.tile([8, V], f32, tag="sm")
                nc.tensor.matmul(x1_ps[:], cs["cw12"][:, 0:8], xt[:],
                                 start=True, stop=True)
                x2_ps = qsm.tile([8, V], f32, tag="sm")
                nc.tensor.matmul(x2_ps[:], cs["cw12"][:, 32:40], xt[:],
                                 start=True, stop=True)
                x1_sb = psmall.tile([8, V], f32, tag="x1s")
                x2_sb = psmall.tile([8, V], f32, tag="x2s")
                nc.scalar.activation(x1_sb[:], x1_ps[:], AF.Identity,
                                     bias=cs["cb1"][:])
                nc.scalar.activation(x2_sb[:], x2_ps[:], AF.Identity,
                                     bias=cs["cb2"][:])
                diff = psmall.tile([8, 625], f32, tag="diff")
                nc.gpsimd.tensor_tensor(
                    diff[:].rearrange("p (v u) -> p v u", v=V),
                    AP(x1_sb[:].tensor, 0, [[V, 8], [0, V], [1, V]]),
                    AP(x2_sb[:].tensor, 0, [[V, 8], [1, V], [0, V]]),
                    ALU.subtract)
                nc.scalar.activation(relp[0:8, :], diff[:], AF.Tanh)

                # ---- rel2 = w4a' @ rel' (A, alpha*b4 folded as rows 8/9) ----
                rel2a = qsm.tile([64, 512], f32, tag="sm")
                nc.tensor.matmul(rel2a[:], cs["cw4a"][:],
                                 relp[:, 0:512], start=True, stop=True)
                rel2b = qsm.tile([64, 113], f32, tag="sm")
                nc.tensor.matmul(rel2b[:], cs["cw4a"][:],
                                 relp[:, 512:625], start=True, stop=True)
                rel3 = rel3_pp[n % 2]
                nc.scalar.activation(rel3[:, 0:512], rel2a[:], AF.Identity,
                                     bias=0.0)
                nc.scalar.activation(rel3[:, 512:625], rel2b[:], AF.Identity,
                                     bias=0.0)
                rel3_t = rel3[:].tensor

                # block-diag build, two clean-AP stages:
                # 1) rel3 [64,625] -> rel3G [125=(og,v), 325=(g,u)]  (13 DMAs,
                #    split across the three DMA-capable queues)
                # 2) rel3G -> BD[og*25+v, g*125+og*25+u]            (5 DMAs)
                # ---- x3 -> y per (v-chunk) ----
                x_bf = pbig.tile([128, TH * V], bf16, tag="xbf")
                nc.vector.tensor_copy(x_bf[:], x_sb[:])
                xbf_t = x_bf[:].tensor
                y_sb = pbig.tile([40, VTH], bf16, tag="y")
                y_t = y_sb[:].tensor
                for (v0, v1) in VCH:
                    vc = v1 - v0
                    x3_ps = qz.tile([40, 448], f32, tag="qz")
                    nc.tensor.matmul(
                        x3_ps[:, 0:vc * TH], cs["cw5x3"][:],
                        AP(xbf_t, v0, [[TH * V, 128], [1, vc], [V, TH]]),
                        start=True, stop=True)
                    # y = (x3 + b5) * rf  (broadcast over v)
                    nc.vector.scalar_tensor_tensor(
                        AP(y_t, v0 * TH, [[VTH, 40], [TH, vc], [1, TH]]),
                        x3_ps[:, 0:vc * TH].rearrange(
                            "p (v t) -> p v t", v=vc),
                        cs["cb5"][:],
                        AP(rf2[:].tensor, 0, [[TH, 40], [0, vc], [1, TH]]),
                        ALU.add, ALU.mult)

                if upto < 5:
                    nc.sync.dma_start(out_d.ap()[n].transpose([1, 0, 2]),
                                      x_sb[:].rearrange("p (a b) -> p a b", a=64))
                    continue
                # ---- yT [ (r,v), t ] via two transpose DMAs ----
                yTA = yTA_pp[n % 2]
                yTB = yTB_pp[n % 2]
                VB = V - VA
                for jj in range(2):
                    nc.scalar.dma_start(
                        AP(yTA[:].tensor, jj * TH, [[T, 8 * VA], [1, TH]]),
                        AP(y_t, jj * 32 * VTH,
                           [[VTH, 8], [TH, VA], [1, TH]]))
                    nc.scalar.dma_start(
                        AP(yTB[:].tensor, jj * TH, [[T, 8 * VB], [1, TH]]),
                        AP(y_t, VA * TH + jj * 32 * VTH,
                           [[VTH, 8], [TH, VB], [1, TH]]))

                if upto < 6:
                    nc.sync.dma_start(out_d.ap()[n].transpose([1, 0, 2]),
                                      x_sb[:].rearrange("p (a b) -> p a b", a=64))
                    continue
                # ---- zT on PE: per group g, two accumulating matmuls ----
                zT = zT_pp[n % 2]
                zT_t = zT[:].tensor
                for g in range(NG):
                    zt_ps = qzt.tile([125, T], f32, tag="qzt")
                    nc.tensor.matmul(zt_ps[:], cs["cwz1"][:, g * 125:(g + 1) * 125],
                                     yTA[:], start=True, stop=False)
                    nc.tensor.matmul(zt_ps[:], cs["cwz2"][:, g * 125:(g + 1) * 125],
                                     yTB[:], start=False, stop=True)
                    dst = AP(zT_t, g * T, [[NG * T, 125], [1, T]])
                    if g % 2 == 0:
                        nc.scalar.activation(dst, zt_ps[:], AF.Identity,
                                             bias=cs["cb3bd"][:, g:g + 1])
                    else:
                        nc.vector.tensor_scalar_add(dst, zt_ps[:],
                                                    cs["cb3bd"][:, g:g + 1])

                if upto < 7:
                    nc.sync.dma_start(
                        AP(out_d.ap().tensor, n * O * T * V,
                           [[1600, 125], [1, 1600]]),
                        zT[:, 0:1600])
                    continue
                rel3g = rel3g_pp[n % 2]
                rel3g_t = rel3g[:].tensor
                RG = NG * 25
                dqs = [nc.sync, nc.gpsimd]
                for g in range(NG):
                    ocnt = 5 if g < NG - 1 else 4
                    dqs[0 if g % 3 == 0 else 1].dma_start(
                        AP(rel3g_t, g * 25,
                           [[RG, ocnt * 25], [1, 25]]),
                        AP(rel3_t, g * 5 * 625,
                           [[625, ocnt], [25, 25], [1, 25]]))
                for og in range(5):
                    gcnt = NG if og < 4 else NG - 1
                    nc.gpsimd.dma_start(
                        AP(bd_t, og * 25 * 1625 + og * 25,
                           [[1625, 25], [125, gcnt], [1, 25]]),
                        AP(rel3g_t, og * 25 * RG,
                           [[RG, 25], [25, gcnt], [1, 25]]))

                if upto < 4:
                    nc.sync.dma_start(out_d.ap()[n].transpose([1, 0, 2]),
                                      x_sb[:].rearrange("p (a b) -> p a b", a=64))
                    continue
                # ---- final: out[t, (og,u)] per group ----
                out_sb = out_pp[n % 2]
                out_t = out_sb[:].tensor
                for q in range(4):
                    glist = list(range(4 * q, min(4 * q + 4, NG)))
                    op_ps = qout.tile([128, 512], f32, tag="outp")
                    for idx, g in enumerate(glist):
                        ko = 125 if g < NG - 1 else 100
                        mo = 125 if g < NG - 1 else 100
                        nc.tensor.matmul(
                            op_ps[:, idx * 128:idx * 128 + mo],
                            AP(zT_t, g * T, [[NG * T, ko], [1, T]]),
                            AP(bd_t, g * 125, [[1625, ko], [1, mo]]),
                            start=True, stop=True)
                    ng = len(glist)
                    mo = 125 if q < 3 else 100
                    src = AP(op_ps[:].tensor, 0,
                             [[512, 128], [128, ng], [1, mo]])
                    dst = AP(out_t, q * 500,
                             [[O * V, 128], [125, ng], [1, mo]])
                    if q < 2:
                        nc.vector.tensor_copy(dst, src)
                    else:
                        nc.scalar.activation(dst, src, AF.Copy)

                # ---- store ----
                nc.sync.dma_start(
                    out_d.ap()[n].transpose([1, 0, 2]), out_sb[:])

    nc.compile()
    return nc


_CACHE = {}


import os
def _get_nc(nb):
    upto = int(os.environ.get("KUPTO", "9"))
    key = (nb, upto)
    if key not in _CACHE:
        _CACHE[key] = build(nb, upto)
    return _CACHE[key]


def kernel(x, A, alpha, w1, b1, w2, b2, w5, b5, w3, b3, w4, b4,
           g_w, g_b, a_w, a_b, bn_g, bn_b, bn_rm, bn_rv, rf_bw,
           _trace=False):
    x = np.asarray(x, np.float32)
    consts = host_consts(np.asarray(A, np.float32), np.asarray(alpha, np.float32),
                         *[np.asarray(t, np.float32) for t in
                           (w1, b1, w2, b2, w5, b5, w3, b3, w4, b4,
                            g_w, g_b, a_w, a_b, bn_g, bn_b, bn_rm, bn_rv, rf_bw)])
    nc = _get_nc(NB)
    in_maps = []
    for i in range(NCORES):
        m = {"x": np.ascontiguousarray(x[i * NB:(i + 1) * NB])}
        m.update(consts)
        in_maps.append(m)
    res = bass_utils.run_bass_kernel_spmd(
        nc, in_maps, core_ids=list(range(NCORES)), trace=_trace)
    out = np.concatenate([res.results[i]["out"] for i in range(NCORES)], axis=0)
    if _trace:
        kernel._last_results = res
    return out.astype(np.float32)
